# revision 17
# baseline (speedup 1.0000x reference)
"""Causal self-attention on 8 TRN2 NeuronCores — all-bf16 datapath.

Reference computation (B=4, T=2048, C=1024, H=16 heads, hd=64):
    qkv = x @ W_attn + b_attn ; split q,k,v ; per-head causal softmax attention
    y = att @ v ; out = y @ W_proj + b_proj
Grader gate: max|diff|/max|expected| < 2e-2.  All-bf16 operands with fp32
PSUM accumulation measure 4.3e-3 in simulation (fp8 anywhere in the q/k, v
or proj paths fails the gate — verified empirically), so every matmul runs
bf16: full PE speed at every free size (fp32r drops to 1/4 rate below
256 columns), FWL halves LDWEIGHTS, and 16-bit DVE modes double the
element-wise rate.

Sharding: core = 2*b + g  (b = batch 0..3, g = head-half 0..1, heads 8g..8g+7).
Each core computes its batch's Q/K/V for its 8 heads, flash-style causal
attention entirely in SBUF, and a partial out^T = Wp_slice^T @ y^T (bf16).
The host sums the two partials per batch and re-transposes to [B,T,C].

Layouts are feature-major (x^T, Q^T, K^T, y^T, out^T) so no transposes are
needed on device.  S^T[k,q] = K^T.T @ Q^T puts softmax on the partition axis;
the denominator comes free from an appended ones-column on V (M=65 matmul).
All host-side tensors are pre-arranged into the exact device layouts
([part128, chunk, free]) so DMAs are dense and no rearranges are needed.

Schedule: emission interleaves three streams so the PE never idles — qkv
projection of t-tile j+1 and earlier out-projections are woven between the
attention steps of q-wave j; within a wave the PV matmul of step k-LAG is
emitted next to the S matmul of step k so the S -> exp -> mask -> PV chain
never stalls the PE.  The two head-parities of a step share one two-bank
PSUM tile so exp+mask run as single instructions.  Softmax normalization
transposes the sums row to [128,4] via a DRAM bounce (a [1,512] DVE
reciprocal is microcoded at ~6.5ns/elem — 3.3us — while [128,4] is 178ns),
takes the reciprocal there in bf16, and broadcasts it back via a stride-0
read; the final wave uses the same path (the old single-partition
reciprocal added ~7us of drain tail).
"""

import numpy as np

B, T, C, H = 4, 2048, 1024, 16
HD = C // H          # 64
HPC = 8              # heads per core
NCORES = 8
TQ = 512             # q tile (free dim / psum bank)
NQT = T // TQ        # 4 q tiles (waves) per batch
NCC = C // 128       # 8 contraction chunks of 128
LAG = 3              # PV lags S by this many attention steps
WQK_PERM = [0, 4, 1, 5, 2, 6, 3, 7]   # m-block order in wqk host layout
WQK_POS = {m: i for i, m in enumerate(WQK_PERM)}

_cache = {}


def _build():
    if "nc" in _cache:
        return _cache["nc"]

    import concourse.bass as bass
    import concourse.tile as tile
    from concourse import bacc, mybir

    F32 = mybir.dt.float32
    BF16 = mybir.dt.bfloat16
    AF = mybir.ActivationFunctionType

    nc = bacc.Bacc("TRN2", target_bir_lowering=False, debug=False,
                   num_devices=NCORES)

    xt_d = nc.dram_tensor("xt", [128, NCC, T], BF16, kind="ExternalInput").ap()
    wqk_d = nc.dram_tensor("wqk", [128, NCC, 1024], BF16,
                           kind="ExternalInput").ap()
    wv_d = nc.dram_tensor("wv", [128, NCC, 512], BF16,
                          kind="ExternalInput").ap()
    wp_d = nc.dram_tensor("wp", [128, 4, 1024], BF16,
                          kind="ExternalInput").ap()
    bqk_d = nc.dram_tensor("bqk", [128, 8], F32, kind="ExternalInput").ap()
    tri_d = nc.dram_tensor("tri", [128, 2, 128], BF16,
                           kind="ExternalInput").ap()
    outp_d = nc.dram_tensor("outp", [C, T], BF16, kind="ExternalOutput").ap()

    with tile.TileContext(nc) as tc:
        import contextlib
        stack = contextlib.ExitStack()
        with stack:
            singles = stack.enter_context(tc.tile_pool(name="singles", bufs=1))
            ps = stack.enter_context(tc.tile_pool(name="ps", space="PSUM",
                                                  bufs=1))
            qpool = stack.enter_context(tc.tile_pool(name="qpool", bufs=2))
            ypool = stack.enter_context(tc.tile_pool(name="ypool", bufs=8))
            xtp = stack.enter_context(tc.tile_pool(name="xtp", bufs=2))
            ppool = stack.enter_context(tc.tile_pool(name="ppool", bufs=6))
            bcp = stack.enter_context(tc.tile_pool(name="bcp", bufs=4))
            ostp = stack.enter_context(tc.tile_pool(name="ostp", bufs=6))
            ystg = stack.enter_context(tc.tile_pool(name="ystg", bufs=4))
            drp = stack.enter_context(tc.tile_pool(name="drp", bufs=16,
                                                   space="DRAM"))

            tri_sb = singles.tile([128, 2, 128], BF16)
            bqk_sb = singles.tile([128, 8], F32)

            warm = singles.tile([1, 4], F32)
            nc.vector.memset(warm, 0.0)
            nc.scalar.activation(warm, warm, AF.Exp)
            nc.gpsimd.dma_start(out=tri_sb, in_=tri_d)
            nc.gpsimd.dma_start(out=bqk_sb, in_=bqk_d)

            # K^T resident: [feat128, pair, t];  V: [t128, kchunk, head, 65]
            k_sb = singles.tile([128, 4, T], BF16)
            v_sb = singles.tile([128, T // 128, HPC, 65], BF16)
            ones_sb = singles.tile([128, (T // 128) * HPC], BF16)
            nc.vector.memset(ones_sb, 1.0)
            nc.vector.tensor_copy(
                v_sb[:, :, :, 64],
                ones_sb.rearrange("p (a b) -> p a b", a=T // 128))

            wqk_sb = singles.tile([128, NCC, 1024], BF16)
            wv_sb = singles.tile([128, NCC, 512], BF16)
            wp_sb = singles.tile([128, 4, 1024], BF16)
            for c in range(NCC):
                nc.gpsimd.dma_start(out=wv_sb[:, c, :], in_=wv_d[:, c, :])
            def load_wp():
                for c in range(4):
                    nc.gpsimd.dma_start(out=wp_sb[:, c, :], in_=wp_d[:, c, :])

            q_tiles = {}   # wave j -> [128, 4, TQ] tile
            y_tiles = {}   # keys (j, cp) psum pair, (j, "sb", cp) sbuf tile

            # ---------- emission closures ----------
            def qkv_groups(tt):
                """13 emission closures for t-tile tt of the projections."""
                xt = [None]

                def load_x():
                    xt[0] = [xtp.tile([128, TQ], BF16, tag=f"xt{c}",
                                      name=f"xt_{tt}_{c}")
                             for c in range(NCC)]
                    xr = xt_d[:, :, tt * TQ:(tt + 1) * TQ]
                    for c in range(NCC):
                        eng = nc.sync if c < 4 else nc.scalar
                        eng.dma_start(out=xt[0][c], in_=xr[:, c, :])
                    if tt == 0:
                        # wqk is laid out m-block-permuted on the host so the
                        # first quarter holds exactly the q/k chunks the first
                        # attention steps need; stream quarters on two queues
                        for qi in range(4):
                            for c in range(NCC):
                                eng = nc.sync if c < 4 else nc.scalar
                                sl = slice(qi * 256, (qi + 1) * 256)
                                eng.dma_start(out=wqk_sb[:, c, sl],
                                              in_=wqk_d[:, c, sl])
                    q_tiles[tt] = qpool.tile([128, 4, TQ], BF16, tag="q",
                                             name=f"q_{tt}")

                def qk_chunk(m):
                    mp = WQK_POS[m]
                    def emit():
                        acc = ps.tile([128, TQ], F32, tag="acc", bufs=2,
                                      name=f"acc_qk_{tt}_{m}")
                        for c in range(NCC):
                            nc.tensor.matmul(
                                acc, wqk_sb[:, c, mp * 128:(mp + 1) * 128],
                                xt[0][c],
                                start=(c == 0), stop=(c == NCC - 1))
                        if m < 4:
                            dst = q_tiles[tt][:, m, :]
                        else:
                            dst = k_sb[:, m - 4, tt * TQ:(tt + 1) * TQ]
                        nc.vector.tensor_scalar_add(dst, acc,
                                                    bqk_sb[:, m:m + 1])
                    return emit

                def v_chunk(v4):
                    def emit():
                        ti = tt * 4 + v4
                        acc = ps.tile([128, TQ], F32, tag="acc", bufs=2,
                                      name=f"acc_v_{tt}_{v4}")
                        for c in range(NCC):
                            nc.tensor.matmul(
                                acc, xt[0][c][:, v4 * 128:(v4 + 1) * 128],
                                wv_sb[:, c, :],
                                start=(c == 0), stop=(c == NCC - 1))
                        nc.vector.tensor_copy(
                            v_sb[:, ti, :, 0:64],
                            acc.rearrange("p (h d) -> p h d", h=HPC))
                    return emit

                # (early, late): early groups emit during wave tt-1 (they
                # gate wave tt's first steps); late groups spill into wave tt.
                early = [load_x]
                early += [v_chunk(v4) for v4 in range(4)]
                early += [qk_chunk(0), qk_chunk(4)]
                late = []
                for cpx in range(1, 4):
                    late += [qk_chunk(cpx), qk_chunk(4 + cpx)]
                return early, late

            def attention_wave(j):
                """Emission closures for q-wave j: pipelined S/exp/PV with
                LAG, plus normalization per head-pair."""
                nkc = 4 * j + 4
                steps = [(cp, i) for cp in range(4) for i in range(nkc)]
                pend = {}

                def emit_S(k):
                    cp, i = steps[k]
                    r = max(0, (i - 4 * j) * 128)
                    s_ps = ps.tile([128, 2, TQ], F32, tag="s", bufs=2,
                                   name=f"s_{j}_{k}")
                    for par in range(2):
                        row0 = 64 * par
                        nc.tensor.matmul(
                            s_ps[:, par, r:TQ],
                            k_sb[row0:row0 + 64, cp, i * 128:(i + 1) * 128],
                            q_tiles[j][row0:row0 + 64, cp, r:TQ],
                            start=True, stop=True, tile_position=(row0, 0))
                    p_sb = ppool.tile([128, 2, TQ], BF16, tag="p",
                                      name=f"p_{j}_{k}")
                    nc.scalar.activation(p_sb[:, :, r:TQ], s_ps[:, :, r:TQ],
                                         AF.Exp)
                    if i >= 4 * j:
                        nc.vector.tensor_mul(p_sb[:, :, r:r + 128],
                                             p_sb[:, :, r:r + 128], tri_sb)
                    pend[k] = (r, p_sb)

                def emit_PV(k):
                    cp, i = steps[k]
                    r, p_sb = pend.pop(k)
                    if i == 0:
                        y_tiles[(j, cp)] = ps.tile(
                            [65, 2, TQ], F32, tag="y", bufs=1,
                            name=f"yps_{j}_{cp}")
                    for par in range(2):
                        nc.tensor.matmul(
                            y_tiles[(j, cp)][:, par, r:TQ],
                            v_sb[:, i, 2 * cp + par, :], p_sb[:, par, r:TQ],
                            start=(i == 0), stop=(i == nkc - 1))
                    if i == nkc - 1:
                        emit_norm(cp)

                def emit_norm(cp):
                    y_ps = y_tiles[(j, cp)]
                    # free the psum banks fast: one copy takes y + sums rows
                    yst = ystg.tile([65, 2, TQ], BF16, tag="yst",
                                    name=f"yst_{j}_{cp}")
                    nc.vector.tensor_copy(yst, y_ps)
                    if j == NQT - 1 and cp == 3:
                        # tail-critical: skip the DRAM bounce. ~18-bit DVE
                        # reciprocal on the single-partition sums row, then
                        # a gpsimd broadcast — no DMA rings on this path.
                        dc = bcp.tile([1, 2, TQ], F32, tag="dc", bufs=1,
                                      name=f"dc_{j}_{cp}")
                        nc.vector.tensor_copy(dc, y_ps[64:65, :, :])
                        rr = bcp.tile([1, 2, TQ], F32, tag="rrf", bufs=1,
                                      name=f"rrf_{j}_{cp}")
                        nc.vector.reciprocal_approx_fast(rr, dc)
                        bcf = bcp.tile([64, 2, TQ], F32, tag="bcf", bufs=1,
                                       name=f"bcf_{j}_{cp}")
                        nc.gpsimd.partition_broadcast(bcf, rr)
                        for par in range(2):
                            row0 = 64 * par
                            nc.vector.tensor_mul(
                                y_tiles[(j, "sb", cp)][row0:row0 + 64, :],
                                yst[0:64, par, :], bcf[:, par, :])
                        return
                    # transpose both sums rows to [128, 8] in one bounce so
                    # the microcoded reciprocal runs lane-parallel
                    s4 = bcp.tile([128, 8], BF16, tag="s4",
                                  name=f"s4_{j}_{cp}")
                    nc.gpsimd.dma_start(out=s4, in_=yst[64:65, :, :])
                    r4 = bcp.tile([128, 8], F32, tag="r4",
                                  name=f"r4_{j}_{cp}")
                    nc.vector.reciprocal(r4, s4)
                    d2 = drp.tile([1, 2 * TQ], F32, tag="d2",
                                  name=f"d2_{j}_{cp}")
                    nc.gpsimd.dma_start(
                        out=bass.AP(tensor=d2.tensor, offset=d2.offset,
                                    ap=[[8, 128], [1, 8]]),
                        in_=r4)
                    for par in range(2):
                        row0 = 64 * par
                        bc = bcp.tile([64, TQ], F32, tag="bc",
                                      name=f"bc_{j}_{cp}_{par}")
                        nc.gpsimd.dma_start(
                            out=bc,
                            in_=bass.AP(tensor=d2.tensor,
                                        offset=d2.offset + par * TQ,
                                        ap=[[0, 64], [1, TQ]]))
                        nc.vector.tensor_mul(
                            y_tiles[(j, "sb", cp)][row0:row0 + 64, :],
                            yst[0:64, par, :], bc)

                def step(k):
                    def emit():
                        if k == 0:
                            for cc in range(4):
                                y_tiles[(j, "sb", cc)] = ypool.tile(
                                    [128, TQ], BF16, tag="ysb",
                                    name=f"y_{j}_{cc}")
                        if k < len(steps):
                            emit_S(k)
                        if k >= LAG:
                            emit_PV(k - LAG)
                    return emit

                return [step(k) for k in range(len(steps) + LAG)]

            def proj_groups(j):
                def chunk(mo):
                    def emit():
                        acc = ps.tile([128, TQ], F32, tag="acc", bufs=2,
                                      name=f"acc_pr_{j}_{mo}")
                        for c in range(4):
                            nc.tensor.matmul(
                                acc, wp_sb[:, c, mo * 128:(mo + 1) * 128],
                                y_tiles[(j, "sb", c)],
                                start=(c == 0), stop=(c == 3))
                        ot = ostp.tile([128, TQ], BF16, tag="ot",
                                       name=f"ot_{j}_{mo}")
                        nc.vector.tensor_copy(ot, acc)
                        eng = nc.sync if mo % 2 == 0 else nc.gpsimd
                        eng.dma_start(
                            out=outp_d[mo * 128:(mo + 1) * 128,
                                       j * TQ:(j + 1) * TQ],
                            in_=ot)
                    return emit
                return [chunk(mo) for mo in range(8)]

            def proj_final(j):
                """Final-wave projection: 8 parallel psum accumulators (the
                attention tags are dead by now, so their bank slots host the
                extra accumulators).  Heads 0-2's contraction streams while
                head-pair 3 is still normalizing; only the 8 c=3 matmuls and
                the drains trail it."""
                accs = {}

                def alloc():
                    accs[0] = ps.tile([128, TQ], F32, tag="acc", bufs=2,
                                      name="fpr_a0")
                    accs[1] = ps.tile([128, TQ], F32, tag="acc", bufs=2,
                                      name="fpr_a1")
                    tA = ps.tile([128, 2, TQ], F32, tag="s", bufs=2,
                                 name="fpr_sA")
                    tB = ps.tile([128, 2, TQ], F32, tag="s", bufs=2,
                                 name="fpr_sB")
                    tY = ps.tile([128, 2, TQ], F32, tag="y", bufs=1,
                                 name="fpr_y")
                    accs[2], accs[3] = tA[:, 0, :], tA[:, 1, :]
                    accs[4], accs[5] = tB[:, 0, :], tB[:, 1, :]
                    accs[6], accs[7] = tY[:, 0, :], tY[:, 1, :]

                def phase_a():
                    alloc()
                    for c in range(3):
                        for mo in range(8):
                            nc.tensor.matmul(
                                accs[mo],
                                wp_sb[:, c, mo * 128:(mo + 1) * 128],
                                y_tiles[(j, "sb", c)],
                                start=(c == 0), stop=False)

                def phase_b(mo):
                    def emit():
                        nc.tensor.matmul(
                            accs[mo], wp_sb[:, 3, mo * 128:(mo + 1) * 128],
                            y_tiles[(j, "sb", 3)],
                            start=False, stop=True)
                        ot = ostp.tile([128, TQ], BF16, tag="ot",
                                       name=f"ot_{j}_{mo}")
                        if mo % 2 == 1:
                            nc.scalar.copy(ot, accs[mo])
                        else:
                            nc.vector.tensor_copy(ot, accs[mo])
                        nc.sync.dma_start(
                            out=outp_d[mo * 128:(mo + 1) * 128,
                                       j * TQ:(j + 1) * TQ],
                            in_=ot)
                    return emit
                return [phase_a] + [phase_b(mo) for mo in range(8)]

            # ---------- interleaved emission ----------
            g0_early, g0_late = qkv_groups(0)
            for fn in g0_early:
                fn()
            spill = list(g0_late)
            for j in range(NQT):
                attn = attention_wave(j)
                others = list(spill)
                spill = []
                if j == 0:
                    others.append(load_wp)
                if j + 1 < NQT:
                    early, late = qkv_groups(j + 1)
                    others += early
                    spill = late
                if j == 2:
                    others += proj_groups(0)
                if j == 3:
                    others += proj_groups(1) + proj_groups(2)
                done_o = 0
                frontier = max(1, (len(attn) * 9) // 10)
                for s, fn in enumerate(attn):
                    fn()
                    want = min(len(others), (s + 1) * len(others) // frontier)
                    while done_o < want:
                        others[done_o]()
                        done_o += 1
                while done_o < len(others):
                    others[done_o]()
                    done_o += 1
            for fn in proj_final(NQT - 1):
                fn()

    nc.compile()
    _cache["nc"] = nc
    return nc


def _prep_inputs(x, W_attn, b_attn, W_proj, b_proj):
    """Host-side sharding: returns in_maps for the 8 cores."""
    import ml_dtypes
    BF = ml_dtypes.bfloat16

    x = np.ascontiguousarray(np.asarray(x, dtype=np.float32))
    W_attn = np.asarray(W_attn, dtype=np.float32)
    b_attn = np.asarray(b_attn, dtype=np.float32)
    W_proj = np.asarray(W_proj, dtype=np.float32)
    b_proj = np.asarray(b_proj, dtype=np.float32)

    bv_full = b_attn[2 * C:3 * C]
    _cache["bout_host"] = (b_proj + bv_full @ W_proj).astype(np.float32)
    tri1 = np.triu(np.ones((128, 128), dtype=np.float32))  # 1 if k<=q
    tri = np.ascontiguousarray(
        np.stack([tri1, tri1], axis=1).astype(BF))        # [128, 2, 128]

    def chunk_rows(a, nch):
        # [nch*128, F] -> [128, nch, F] bf16 in device layout
        f = a.shape[1]
        return np.ascontiguousarray(
            a.reshape(nch, 128, f).transpose(1, 0, 2).astype(BF))

    xts = [chunk_rows(np.ascontiguousarray(x[b].T), NCC) for b in range(B)]
    per_g = []
    for g in range(2):
        sl = slice(512 * g, 512 * (g + 1))
        wq = W_attn[:, 0:C][:, sl] * (1.0 / np.sqrt(HD))
        wk = W_attn[:, C:2 * C][:, sl]
        wv = W_attn[:, 2 * C:3 * C][:, sl]
        bq = b_attn[0:C][sl] * (1.0 / np.sqrt(HD))
        bk = b_attn[C:2 * C][sl]
        wp = W_proj[sl, :]
        wqk_log = np.concatenate([wq, wk], axis=1)
        wqk_perm = np.concatenate(
            [wqk_log[:, m * 128:(m + 1) * 128] for m in WQK_PERM], axis=1)
        per_g.append({
            "wqk": chunk_rows(wqk_perm, NCC),
            "wv": chunk_rows(wv, NCC),
            "wp": chunk_rows(wp, 4),
            "bqk": np.ascontiguousarray(
                np.concatenate([bq, bk]).reshape(8, 128).T.astype(np.float32)),
        })

    in_maps = []
    for b in range(B):
        for g in range(2):
            m = dict(per_g[g])
            m["xt"] = xts[b]
            m["tri"] = tri
            in_maps.append(m)
    return in_maps


def run_sharded(x, W_attn, b_attn, W_proj, b_proj, trace=False):
    """Run on 8 cores; returns (output [B,T,C], BassKernelResults)."""
    from concourse.bass_utils import run_bass_kernel_spmd

    nc = _build()
    in_maps = _prep_inputs(x, W_attn, b_attn, W_proj, b_proj)
    res = run_bass_kernel_spmd(nc, in_maps, list(range(NCORES)), trace=trace)
    outs = [np.asarray(res.results[i]["outp"], dtype=np.float32)
            for i in range(NCORES)]
    bout = _cache["bout_host"]
    out = np.empty((B, T, C), dtype=np.float32)
    for b in range(B):
        out[b] = (outs[2 * b] + outs[2 * b + 1]).T + bout
    return out, res


def kernel(x, W_attn, b_attn, W_proj, b_proj):
    out, _ = run_sharded(x, W_attn, b_attn, W_proj, b_proj, trace=False)
    return out


# revision 26
# speedup vs baseline: 1.0204x; 1.0204x over previous
"""Causal self-attention on 8 TRN2 NeuronCores — all-bf16 datapath.

Reference computation (B=4, T=2048, C=1024, H=16 heads, hd=64):
    qkv = x @ W_attn + b_attn ; split q,k,v ; per-head causal softmax attention
    y = att @ v ; out = y @ W_proj + b_proj
Grader gate: max|diff|/max|expected| < 2e-2.  All-bf16 operands with fp32
PSUM accumulation measure 4.3e-3 in simulation (fp8 anywhere in the q/k, v
or proj paths fails the gate — verified empirically), so every matmul runs
bf16: full PE speed at every free size (fp32r drops to 1/4 rate below
256 columns), FWL halves LDWEIGHTS, and 16-bit DVE modes double the
element-wise rate.

Sharding: core = 2*b + g  (b = batch 0..3, g = head-half 0..1, heads 8g..8g+7).
Each core computes its batch's Q/K/V for its 8 heads, flash-style causal
attention entirely in SBUF, and a partial out^T = Wp_slice^T @ y^T (bf16).
The host sums the two partials per batch and re-transposes to [B,T,C].

Layouts are feature-major (x^T, Q^T, K^T, y^T, out^T) so no transposes are
needed on device.  S^T[k,q] = K^T.T @ Q^T puts softmax on the partition axis;
the denominator comes free from an appended ones-column on V (M=65 matmul).
All host-side tensors are pre-arranged into the exact device layouts
([part128, chunk, free]) so DMAs are dense and no rearranges are needed.

Schedule: emission interleaves three streams so the PE never idles — qkv
projection of t-tile j+1 and earlier out-projections are woven between the
attention steps of q-wave j; within a wave the PV matmul of step k-LAG is
emitted next to the S matmul of step k so the S -> exp -> mask -> PV chain
never stalls the PE.  The two head-parities of a step share one two-bank
PSUM tile so exp+mask run as single instructions.  Softmax normalization
transposes the sums row to [128,4] via a DRAM bounce (a [1,512] DVE
reciprocal is microcoded at ~6.5ns/elem — 3.3us — while [128,4] is 178ns),
takes the reciprocal there in bf16, and broadcasts it back via a stride-0
read; the final wave uses the same path (the old single-partition
reciprocal added ~7us of drain tail).
"""

import numpy as np

B, T, C, H = 4, 2048, 1024, 16
HD = C // H          # 64
HPC = 8              # heads per core
NCORES = 8
TQ = 512             # q tile (free dim / psum bank)
NQT = T // TQ        # 4 q tiles (waves) per batch
NCC = C // 128       # 8 contraction chunks of 128
LAG = 3              # PV lags S by this many attention steps
WQK_PERM = [0, 4, 1, 5, 2, 6, 3, 7]   # m-block order in wqk host layout
WQK_POS = {m: i for i, m in enumerate(WQK_PERM)}

_cache = {}


def _build():
    if "nc" in _cache:
        return _cache["nc"]

    import concourse.bass as bass
    import concourse.tile as tile
    from concourse import bacc, mybir

    F32 = mybir.dt.float32
    F32R = mybir.dt.float32r
    BF16 = mybir.dt.bfloat16
    AF = mybir.ActivationFunctionType

    nc = bacc.Bacc("TRN2", target_bir_lowering=False, debug=False,
                   num_devices=NCORES)

    xt_d = nc.dram_tensor("xt", [128, NCC, T], BF16, kind="ExternalInput").ap()
    wqk_d = nc.dram_tensor("wqk", [128, NCC, 1024], BF16,
                           kind="ExternalInput").ap()
    wv_d = nc.dram_tensor("wv", [128, NCC, 512], BF16,
                          kind="ExternalInput").ap()
    wp_d = nc.dram_tensor("wp", [128, 4, 1024], BF16,
                          kind="ExternalInput").ap()
    bqk_d = nc.dram_tensor("bqk", [128, 8], F32, kind="ExternalInput").ap()
    tri_d = nc.dram_tensor("tri", [128, 2, 128], BF16,
                           kind="ExternalInput").ap()
    outp_d = nc.dram_tensor("outp", [C, T], BF16, kind="ExternalOutput").ap()

    with tile.TileContext(nc) as tc:
        import contextlib
        stack = contextlib.ExitStack()
        with stack:
            singles = stack.enter_context(tc.tile_pool(name="singles", bufs=1))
            ps = stack.enter_context(tc.tile_pool(name="ps", space="PSUM",
                                                  bufs=1))
            qpool = stack.enter_context(tc.tile_pool(name="qpool", bufs=2))
            ypool = stack.enter_context(tc.tile_pool(name="ypool", bufs=8))
            xtp = stack.enter_context(tc.tile_pool(name="xtp", bufs=2))
            ppool = stack.enter_context(tc.tile_pool(name="ppool", bufs=6))
            bcp = stack.enter_context(tc.tile_pool(name="bcp", bufs=4))
            ostp = stack.enter_context(tc.tile_pool(name="ostp", bufs=6))
            ystg = stack.enter_context(tc.tile_pool(name="ystg", bufs=4))
            drp = stack.enter_context(tc.tile_pool(name="drp", bufs=16,
                                                   space="DRAM"))

            tri_sb = singles.tile([128, 2, 128], BF16)
            bqk_sb = singles.tile([128, 8], F32)

            warm = singles.tile([1, 4], F32)
            nc.vector.memset(warm, 0.0)
            nc.scalar.activation(warm, warm, AF.Exp)
            nc.gpsimd.dma_start(out=tri_sb, in_=tri_d)
            nc.gpsimd.dma_start(out=bqk_sb, in_=bqk_d)

            # K^T resident: [feat128, pair, t];  V: [t128, kchunk, head, 65]
            k_sb = singles.tile([128, 4, T], BF16)
            v_sb = singles.tile([128, T // 128, HPC, 65], BF16)
            ones_sb = singles.tile([128, (T // 128) * HPC], BF16)
            nc.vector.memset(ones_sb, 1.0)
            ones1 = singles.tile([1, 64], F32)
            nc.vector.memset(ones1, 1.0)
            nc.vector.tensor_copy(
                v_sb[:, :, :, 64],
                ones_sb.rearrange("p (a b) -> p a b", a=T // 128))

            wqk_sb = singles.tile([128, NCC, 1024], BF16)
            wv_sb = singles.tile([128, NCC, 512], BF16)
            wp_sb = singles.tile([128, 4, 1024], BF16)
            for c in range(NCC):
                nc.gpsimd.dma_start(out=wv_sb[:, c, :], in_=wv_d[:, c, :])
            def load_wp():
                for c in range(4):
                    nc.gpsimd.dma_start(out=wp_sb[:, c, :], in_=wp_d[:, c, :])

            q_tiles = {}   # wave j -> [128, 4, TQ] tile
            y_tiles = {}   # keys (j, cp) psum pair, (j, "sb", cp) sbuf tile

            # ---------- emission closures ----------
            def qkv_groups(tt):
                """13 emission closures for t-tile tt of the projections."""
                xt = [None]

                def load_x():
                    xt[0] = [xtp.tile([128, TQ], BF16, tag=f"xt{c}",
                                      name=f"xt_{tt}_{c}")
                             for c in range(NCC)]
                    xr = xt_d[:, :, tt * TQ:(tt + 1) * TQ]
                    for c in range(NCC):
                        # scalar queue only helps at t=0 (it is exp-busy later)
                        eng = nc.sync if (c < 4 or tt > 0) else nc.scalar
                        eng.dma_start(out=xt[0][c], in_=xr[:, c, :])
                    if tt == 0:
                        # wqk is laid out m-block-permuted on the host so the
                        # first quarter holds exactly the q/k chunks the first
                        # attention steps need; stream quarters on two queues
                        for qi in range(4):
                            for c in range(NCC):
                                eng = nc.sync if c < 4 else nc.scalar
                                sl = slice(qi * 256, (qi + 1) * 256)
                                eng.dma_start(out=wqk_sb[:, c, sl],
                                              in_=wqk_d[:, c, sl])
                    q_tiles[tt] = qpool.tile([128, 4, TQ], BF16, tag="q",
                                             name=f"q_{tt}")

                def qk_chunk(m):
                    mp = WQK_POS[m]
                    def emit():
                        acc = ps.tile([128, TQ], F32, tag="acc", bufs=2,
                                      name=f"acc_qk_{tt}_{m}")
                        for c in range(NCC):
                            nc.tensor.matmul(
                                acc, wqk_sb[:, c, mp * 128:(mp + 1) * 128],
                                xt[0][c],
                                start=(c == 0), stop=(c == NCC - 1))
                        if m < 4:
                            dst = q_tiles[tt][:, m, :]
                        else:
                            dst = k_sb[:, m - 4, tt * TQ:(tt + 1) * TQ]
                        nc.vector.tensor_scalar_add(dst, acc,
                                                    bqk_sb[:, m:m + 1])
                    return emit

                def v_chunk(v4):
                    def emit():
                        ti = tt * 4 + v4
                        acc = ps.tile([128, TQ], F32, tag="acc", bufs=2,
                                      name=f"acc_v_{tt}_{v4}")
                        for c in range(NCC):
                            nc.tensor.matmul(
                                acc, xt[0][c][:, v4 * 128:(v4 + 1) * 128],
                                wv_sb[:, c, :],
                                start=(c == 0), stop=(c == NCC - 1))
                        nc.vector.tensor_copy(
                            v_sb[:, ti, :, 0:64],
                            acc.rearrange("p (h d) -> p h d", h=HPC))
                    return emit

                # (early, late): early groups emit during wave tt-1 (they
                # gate wave tt's first steps); late groups spill into wave tt.
                # qk before v so the q/k bias-adds aren't queued behind the
                # v-copies on the Vector engine (wave starts gate on q/k).
                early = [load_x, qk_chunk(0), qk_chunk(4)]
                early += [v_chunk(v4) for v4 in range(4)]
                late = []
                for cpx in range(1, 4):
                    late += [qk_chunk(cpx), qk_chunk(4 + cpx)]
                return early, late

            def attention_wave(j):
                """Emission closures for q-wave j: pipelined S/exp/PV with
                LAG, plus normalization per head-pair."""
                nkc = 4 * j + 4
                steps = [(cp, i) for cp in range(4) for i in range(nkc)]
                pend = {}

                def emit_S(k):
                    cp, i = steps[k]
                    r = max(0, (i - 4 * j) * 128)
                    s_ps = ps.tile([128, 2, TQ], F32, tag="s", bufs=2,
                                   name=f"s_{j}_{k}")
                    for par in range(2):
                        row0 = 64 * par
                        nc.tensor.matmul(
                            s_ps[:, par, r:TQ],
                            k_sb[row0:row0 + 64, cp, i * 128:(i + 1) * 128],
                            q_tiles[j][row0:row0 + 64, cp, r:TQ],
                            start=True, stop=True, tile_position=(row0, 0))
                    p_sb = ppool.tile([128, 2, TQ], BF16, tag="p",
                                      name=f"p_{j}_{k}")
                    nc.scalar.activation(p_sb[:, :, r:TQ], s_ps[:, :, r:TQ],
                                         AF.Exp)
                    if i >= 4 * j:
                        nc.vector.tensor_mul(p_sb[:, :, r:r + 128],
                                             p_sb[:, :, r:r + 128], tri_sb)
                    pend[k] = (r, p_sb)

                def emit_PV(k):
                    cp, i = steps[k]
                    r, p_sb = pend.pop(k)
                    if i == 0:
                        y_tiles[(j, cp)] = ps.tile(
                            [65, 2, TQ], F32, tag="y", bufs=1,
                            name=f"yps_{j}_{cp}")
                    for par in range(2):
                        nc.tensor.matmul(
                            y_tiles[(j, cp)][:, par, r:TQ],
                            v_sb[:, i, 2 * cp + par, :], p_sb[:, par, r:TQ],
                            start=(i == 0), stop=(i == nkc - 1))
                    if i == nkc - 1:
                        emit_norm(cp)

                def emit_norm(cp):
                    y_ps = y_tiles[(j, cp)]
                    # free the psum banks fast: one copy takes y + sums rows
                    yst = ystg.tile([65, 2, TQ], BF16, tag="yst",
                                    name=f"yst_{j}_{cp}")
                    nc.vector.tensor_copy(yst, y_ps)
                    if j == NQT - 1 and cp == 3:
                        # tail-critical: no DMA and no gpsimd here (a Q7 op
                        # would first drain the queue's DMA backlog, ~13us).
                        # ~18-bit DVE reciprocal on the sums row, then a
                        # contraction-1 PE matmul broadcasts it down the
                        # partitions into a borrowed psum half (norm3[cp3]).
                        dc = bcp.tile([1, 2, TQ], F32, tag="dc", bufs=1,
                                      name=f"dc_{j}_{cp}")
                        nc.vector.tensor_copy(dc, y_ps[64:65, :, :])
                        rr = bcp.tile([1, 2, TQ], F32, tag="rrf", bufs=1,
                                      name=f"rrf_{j}_{cp}")
                        nc.vector.reciprocal_approx_fast(rr, dc)
                        tY = ps.tile([128, 2, TQ], F32, tag="y", bufs=1,
                                     name="fpr_y")
                        y_tiles["bc_ps"] = tY
                        for par in range(2):
                            row0 = 64 * par
                            nc.tensor.matmul(
                                tY[row0:row0 + 64, 1, :], ones1,
                                rr[:, par, :], start=True, stop=True,
                                tile_position=(0, row0))
                            nc.vector.tensor_mul(
                                y_tiles[(j, "sb", cp)][row0:row0 + 64, :],
                                yst[0:64, par, :], tY[row0:row0 + 64, 1, :])
                        return
                    # transpose both sums rows to [128, 8] in one bounce so
                    # the microcoded reciprocal runs lane-parallel
                    s4 = bcp.tile([128, 8], BF16, tag="s4",
                                  name=f"s4_{j}_{cp}")
                    nc.gpsimd.dma_start(out=s4, in_=yst[64:65, :, :])
                    r4 = bcp.tile([128, 8], F32, tag="r4",
                                  name=f"r4_{j}_{cp}")
                    nc.vector.reciprocal(r4, s4)
                    d2 = drp.tile([1, 2 * TQ], F32, tag="d2",
                                  name=f"d2_{j}_{cp}")
                    nc.gpsimd.dma_start(
                        out=bass.AP(tensor=d2.tensor, offset=d2.offset,
                                    ap=[[8, 128], [1, 8]]),
                        in_=r4)
                    for par in range(2):
                        row0 = 64 * par
                        bc = bcp.tile([64, TQ], F32, tag="bc",
                                      name=f"bc_{j}_{cp}_{par}")
                        nc.gpsimd.dma_start(
                            out=bc,
                            in_=bass.AP(tensor=d2.tensor,
                                        offset=d2.offset + par * TQ,
                                        ap=[[0, 64], [1, TQ]]))
                        nc.vector.tensor_mul(
                            y_tiles[(j, "sb", cp)][row0:row0 + 64, :],
                            yst[0:64, par, :], bc)

                def step(k):
                    def emit():
                        if k == 0:
                            for cc in range(4):
                                y_tiles[(j, "sb", cc)] = ypool.tile(
                                    [128, TQ], BF16, tag="ysb",
                                    name=f"y_{j}_{cc}")
                        if k < len(steps):
                            emit_S(k)
                        if k >= LAG:
                            emit_PV(k - LAG)
                    return emit

                return [step(k) for k in range(len(steps) + LAG)]

            def proj_groups(j):
                def chunk(mo):
                    def emit():
                        acc = ps.tile([128, TQ], F32, tag="acc", bufs=2,
                                      name=f"acc_pr_{j}_{mo}")
                        for c in range(4):
                            nc.tensor.matmul(
                                acc, wp_sb[:, c, mo * 128:(mo + 1) * 128],
                                y_tiles[(j, "sb", c)],
                                start=(c == 0), stop=(c == 3))
                        ot = ostp.tile([128, TQ], BF16, tag="ot",
                                       name=f"ot_{j}_{mo}")
                        nc.vector.tensor_copy(ot, acc)
                        eng = nc.sync if mo % 2 == 0 else nc.gpsimd
                        eng.dma_start(
                            out=outp_d[mo * 128:(mo + 1) * 128,
                                       j * TQ:(j + 1) * TQ],
                            in_=ot)
                    return emit
                return [chunk(mo) for mo in range(8)]

            def proj_final(j):
                """Final-wave projection: 7 parallel psum accumulators start
                streaming heads 0-2's contraction while head-pair 3 is still
                normalizing (the attention tags are dead by now, so their
                bank slots host the accumulators; one psum half is lent to
                the cp3 norm broadcast and hosts mo7 afterwards).  Only 11
                matmuls and the drains trail the last normalization."""
                accs = {}

                def drain(mo):
                    ot = ostp.tile([128, TQ], BF16, tag="ot",
                                   name=f"ot_{j}_{mo}")
                    if mo % 2 == 1:
                        nc.scalar.copy(ot, accs[mo])
                    else:
                        nc.vector.tensor_copy(ot, accs[mo])
                    nc.sync.dma_start(
                        out=outp_d[mo * 128:(mo + 1) * 128,
                                   j * TQ:(j + 1) * TQ],
                        in_=ot)

                def phase_a():
                    accs[0] = ps.tile([128, TQ], F32, tag="acc", bufs=2,
                                      name="fpr_a0")
                    accs[1] = ps.tile([128, TQ], F32, tag="acc", bufs=2,
                                      name="fpr_a1")
                    tA = ps.tile([128, 2, TQ], F32, tag="s", bufs=2,
                                 name="fpr_sA")
                    tB = ps.tile([128, 2, TQ], F32, tag="s", bufs=2,
                                 name="fpr_sB")
                    accs[2], accs[3] = tA[:, 0, :], tA[:, 1, :]
                    accs[4], accs[5] = tB[:, 0, :], tB[:, 1, :]
                    accs[6] = y_tiles["bc_ps"][:, 0, :]
                    for c in range(3):
                        for mo in range(7):
                            nc.tensor.matmul(
                                accs[mo],
                                wp_sb[:, c, mo * 128:(mo + 1) * 128],
                                y_tiles[(j, "sb", c)],
                                start=(c == 0), stop=False)

                def phase_b():
                    # mo7 reuses the broadcast half-bank once the norm
                    # muls have read it
                    accs[7] = y_tiles["bc_ps"][:, 1, :]
                    for c in range(3):
                        nc.tensor.matmul(
                            accs[7], wp_sb[:, c, 7 * 128:8 * 128],
                            y_tiles[(j, "sb", c)],
                            start=(c == 0), stop=False)
                    for mo in range(8):
                        nc.tensor.matmul(
                            accs[mo], wp_sb[:, 3, mo * 128:(mo + 1) * 128],
                            y_tiles[(j, "sb", 3)],
                            start=False, stop=True)
                        drain(mo)
                return [phase_a, phase_b]

            # ---------- interleaved emission ----------
            g0_early, g0_late = qkv_groups(0)
            for fn in g0_early:
                fn()
            spill = list(g0_late)
            for j in range(NQT):
                attn = attention_wave(j)
                others = list(spill)
                spill = []
                if j == 0:
                    others.append(load_wp)
                if j + 1 < NQT:
                    early, late = qkv_groups(j + 1)
                    others += early
                    spill = late
                if j == 2:
                    others += proj_groups(0)
                if j == 3:
                    others += proj_groups(1) + proj_groups(2)
                done_o = 0
                frontier = max(1, (len(attn) * 9) // 10)
                for s, fn in enumerate(attn):
                    fn()
                    want = min(len(others), (s + 1) * len(others) // frontier)
                    while done_o < want:
                        others[done_o]()
                        done_o += 1
                while done_o < len(others):
                    others[done_o]()
                    done_o += 1
            for fn in proj_final(NQT - 1):
                fn()

    nc.compile()
    _cache["nc"] = nc
    return nc


def _prep_inputs(x, W_attn, b_attn, W_proj, b_proj):
    """Host-side sharding: returns in_maps for the 8 cores."""
    import ml_dtypes
    BF = ml_dtypes.bfloat16

    x = np.ascontiguousarray(np.asarray(x, dtype=np.float32))
    W_attn = np.asarray(W_attn, dtype=np.float32)
    b_attn = np.asarray(b_attn, dtype=np.float32)
    W_proj = np.asarray(W_proj, dtype=np.float32)
    b_proj = np.asarray(b_proj, dtype=np.float32)

    bv_full = b_attn[2 * C:3 * C]
    _cache["bout_host"] = (b_proj + bv_full @ W_proj).astype(np.float32)
    tri1 = np.triu(np.ones((128, 128), dtype=np.float32))  # 1 if k<=q
    tri = np.ascontiguousarray(
        np.stack([tri1, tri1], axis=1).astype(BF))        # [128, 2, 128]

    def chunk_rows(a, nch):
        # [nch*128, F] -> [128, nch, F] bf16 in device layout
        f = a.shape[1]
        return np.ascontiguousarray(
            a.reshape(nch, 128, f).transpose(1, 0, 2).astype(BF))

    xts = [chunk_rows(np.ascontiguousarray(x[b].T), NCC) for b in range(B)]
    per_g = []
    for g in range(2):
        sl = slice(512 * g, 512 * (g + 1))
        wq = W_attn[:, 0:C][:, sl] * (1.0 / np.sqrt(HD))
        wk = W_attn[:, C:2 * C][:, sl]
        wv = W_attn[:, 2 * C:3 * C][:, sl]
        bq = b_attn[0:C][sl] * (1.0 / np.sqrt(HD))
        bk = b_attn[C:2 * C][sl]
        wp = W_proj[sl, :]
        wqk_log = np.concatenate([wq, wk], axis=1)
        wqk_perm = np.concatenate(
            [wqk_log[:, m * 128:(m + 1) * 128] for m in WQK_PERM], axis=1)
        per_g.append({
            "wqk": chunk_rows(wqk_perm, NCC),
            "wv": chunk_rows(wv, NCC),
            "wp": chunk_rows(wp, 4),
            "bqk": np.ascontiguousarray(
                np.concatenate([bq, bk]).reshape(8, 128).T.astype(np.float32)),
        })

    in_maps = []
    for b in range(B):
        for g in range(2):
            m = dict(per_g[g])
            m["xt"] = xts[b]
            m["tri"] = tri
            in_maps.append(m)
    return in_maps


def run_sharded(x, W_attn, b_attn, W_proj, b_proj, trace=False):
    """Run on 8 cores; returns (output [B,T,C], BassKernelResults)."""
    from concourse.bass_utils import run_bass_kernel_spmd

    nc = _build()
    in_maps = _prep_inputs(x, W_attn, b_attn, W_proj, b_proj)
    res = run_bass_kernel_spmd(nc, in_maps, list(range(NCORES)), trace=trace)
    outs = [np.asarray(res.results[i]["outp"], dtype=np.float32)
            for i in range(NCORES)]
    bout = _cache["bout_host"]
    out = np.empty((B, T, C), dtype=np.float32)
    for b in range(B):
        out[b] = (outs[2 * b] + outs[2 * b + 1]).T + bout
    return out, res


def kernel(x, W_attn, b_attn, W_proj, b_proj):
    out, _ = run_sharded(x, W_attn, b_attn, W_proj, b_proj, trace=False)
    return out
